# revision 1
# baseline (speedup 1.0000x reference)
"""TP-8 decode attention kernel for TRN2 (Bass/Tile).

Shards the 8 KV heads (and their 2 q heads each) across 8 NeuronCores.
Per core: qkv projection (1/8 of columns), RoPE, scores vs its K-cache
shard, softmax with new-token fixup, probs@V, out-proj partial (1/8 of
rows). Host sums the 8 partial outputs (the out_proj all-reduce).

All compute-engine accesses keep partition base 0 (HW quadrant rule):
per-batch score rows are produced by accumulating batch-masked qT
matmuls into one [16, N] PSUM tile; the V product is computed
transposed (V tiles as lhsT) so outputs land on d-partitions.
"""

import sys

sys.path.insert(0, "/opt/trn_rl_repo")

import numpy as np

B, S, C = 8, 1, 4096
DIM = 3072
HQ, HKV, HD = 16, 8, 256
REP = HQ // HKV  # 2
NCORES = 8
SCALE = HD ** (-0.5)


def build_bass():
    import concourse.bass as bass  # noqa: F401
    import concourse.mybir as mybir
    import concourse.tile as tile
    from concourse import bacc
    from contextlib import ExitStack

    f32 = mybir.dt.float32
    Alu = mybir.AluOpType
    Act = mybir.ActivationFunctionType

    nc = bacc.Bacc("TRN2", target_bir_lowering=False, debug=False,
                   num_devices=NCORES)

    xT = nc.dram_tensor("xT", [128, 24 * B], f32, kind="ExternalInput").ap()
    wqkv = nc.dram_tensor("wqkv", [24, 128, 1024], f32, kind="ExternalInput").ap()
    kT = nc.dram_tensor("kT", [B, 2, 128, C], f32, kind="ExternalInput").ap()
    vv = nc.dram_tensor("vv", [B, 8, 128, 1024], f32, kind="ExternalInput").ap()
    wout = nc.dram_tensor("wout", [4, 128, DIM], f32, kind="ExternalInput").ap()
    fm = nc.dram_tensor("fm", [16, C], f32, kind="ExternalInput").ap()
    cs4 = nc.dram_tensor("cs4", [128, 4], f32, kind="ExternalInput").ap()
    ident = nc.dram_tensor("ident", [128, 128], f32, kind="ExternalInput").ap()
    mkv = nc.dram_tensor("mkv", [16, 1], f32, kind="ExternalInput").ap()
    dup = nc.dram_tensor("dup", [B, 16], f32, kind="ExternalInput").ap()
    cmask = nc.dram_tensor("cmask", [128, B * 16], f32, kind="ExternalInput").ap()
    ones8 = nc.dram_tensor("ones8", [1, 128], f32, kind="ExternalInput").ap()
    y = nc.dram_tensor("y", [B, DIM], f32, kind="ExternalOutput").ap()

    with tile.TileContext(nc) as tc, ExitStack() as stk:
        io = stk.enter_context(tc.tile_pool(name="io", bufs=1))
        tmpp = stk.enter_context(tc.tile_pool(name="tmp", bufs=4))
        wp = stk.enter_context(tc.tile_pool(name="wp", bufs=3))
        kvp = stk.enter_context(tc.tile_pool(name="kvp", bufs=20))
        wop = stk.enter_context(tc.tile_pool(name="wop", bufs=3))
        ps = stk.enter_context(tc.tile_pool(name="ps", bufs=8, space="PSUM"))

        # ---- small constants ----
        xT_sb = io.tile([128, 24 * B], f32, tag="xT")
        nc.sync.dma_start(xT_sb[:], xT)
        fm_sb = io.tile([16, C], f32, tag="fm")
        nc.sync.dma_start(fm_sb[:], fm)
        cs_sb = io.tile([128, 4], f32, tag="cs")
        nc.sync.dma_start(cs_sb[:], cs4)
        id_sb = io.tile([128, 128], f32, tag="id")
        nc.sync.dma_start(id_sb[:], ident)
        mkv_sb = io.tile([16, 1], f32, tag="mkv")
        nc.sync.dma_start(mkv_sb[:], mkv)
        dup_sb = io.tile([B, 16], f32, tag="dup")
        nc.sync.dma_start(dup_sb[:], dup)
        cm_sb = io.tile([128, B * 16], f32, tag="cm")
        nc.sync.dma_start(cm_sb[:], cmask)
        on_sb = io.tile([1, 128], f32, tag="on")
        nc.sync.dma_start(on_sb[:], ones8)
        cos_s, sin_s = cs_sb[:, 0:1], cs_sb[:, 1:2]
        cos_p, sin_p = cs_sb[:, 2:3], cs_sb[:, 3:4]

        # ---- phase 1: qkvT = W_shard^T @ x^T  (8 chunks of [128, B]) ----
        chunks = [ps.tile([128, B], f32, tag="ps", name=f"qkvT{i}")
                  for i in range(8)]
        for t in range(24):
            wt = wp.tile([128, 1024], f32, tag="w")
            nc.sync.dma_start(wt[:], wqkv[t])
            for c in range(8):
                nc.tensor.matmul(chunks[c][:], wt[:, c * 128:(c + 1) * 128],
                                 xT_sb[:, t * B:(t + 1) * B],
                                 start=(t == 0), stop=(t == 23))

        # ---- rope ----
        qTh = [io.tile([128, 16], f32, tag=f"qTh{h}", name=f"qTh{h}")
               for h in range(2)]
        knT = [io.tile([128, B], f32, tag=f"knT{h}", name=f"knT{h}")
               for h in range(2)]

        def rope(c1, c2, cosa, sina, out1, out2):
            ta = tmpp.tile([128, B], f32, tag="tmp", name="ta")
            tb = tmpp.tile([128, B], f32, tag="tmp", name="tb")
            nc.vector.tensor_scalar_mul(ta[:], c1, cosa)
            nc.vector.tensor_scalar_mul(tb[:], c2, sina)
            nc.vector.tensor_tensor(out1, ta[:], tb[:], op=Alu.subtract)
            tc_ = tmpp.tile([128, B], f32, tag="tmp", name="tc_")
            td = tmpp.tile([128, B], f32, tag="tmp", name="td")
            nc.vector.tensor_scalar_mul(tc_[:], c1, sina)
            nc.vector.tensor_scalar_mul(td[:], c2, cosa)
            nc.vector.tensor_tensor(out2, tc_[:], td[:], op=Alu.add)

        for r in range(2):
            o1 = qTh[0][:].rearrange("p (b r) -> p r b", r=2)[:, r]
            o2 = qTh[1][:].rearrange("p (b r) -> p r b", r=2)[:, r]
            rope(chunks[2 * r][:], chunks[2 * r + 1][:], cos_s, sin_s, o1, o2)
        rope(chunks[4][:], chunks[5][:], cos_p, sin_p, knT[0][:], knT[1][:])

        # batch-masked qT copies: qThM[b][h] has only cols 2b,2b+1 nonzero
        qThM = [[io.tile([128, 16], f32, tag=f"qM{b}_{h}", name=f"qM{b}_{h}")
                 for h in range(2)] for b in range(B)]
        for b in range(B):
            for h in range(2):
                nc.vector.tensor_tensor(qThM[b][h][:], qTh[h][:],
                                        cm_sb[:, b * 16:(b + 1) * 16],
                                        op=Alu.mult)

        # v_newT chunks -> sbuf [128(d),B] and row-major [B,128] halves
        vnT = [io.tile([128, B], f32, tag=f"vnT{h}", name=f"vnT{h}")
               for h in range(2)]
        vn_row = [io.tile([B, 128], f32, tag=f"vnr{h}", name=f"vnr{h}")
                  for h in range(2)]
        for h in range(2):
            nc.scalar.copy(vnT[h][:], chunks[6 + h][:])
            pvt = ps.tile([B, 128], f32, tag="ps")
            nc.tensor.transpose(pvt[:], vnT[h][:], id_sb[:])
            nc.scalar.copy(vn_row[h][:], pvt[:])

        # ---- s_new[16,1] via masked accumulation (+ mask[kv]) ----
        psn = ps.tile([16, 1], f32, tag="ps")
        for b in range(B):
            for h in range(2):
                nc.tensor.matmul(psn[:], qThM[b][h][:], knT[h][:, b:b + 1],
                                 start=(b == 0 and h == 0),
                                 stop=(b == B - 1 and h == 1))
        s_new = io.tile([16, 1], f32, tag="snew")
        nc.vector.tensor_scalar_add(s_new[:], psn[:], mkv_sb[:, 0:1])

        # ---- phase 2: scores, masked-accumulated over batches ----
        scores = io.tile([16, C], f32, tag="scores")
        mparts = io.tile([16, 8], f32, tag="mparts")
        for g in range(4):  # c-range groups of 1024
            kt = {}
            for b in range(B):
                for h in range(2):
                    kk = kvp.tile([128, 1024], f32, tag="kv",
                                  name=f"k{g}_{b}_{h}")
                    nc.sync.dma_start(kk[:],
                                      kT[b, h][:, g * 1024:(g + 1) * 1024])
                    kt[(b, h)] = kk
            for j in range(2):
                pch = ps.tile([16, 512], f32, tag="ps")
                first = True
                for b in range(B):
                    for h in range(2):
                        nc.tensor.matmul(pch[:], qThM[b][h][:],
                                         kt[(b, h)][:, j * 512:(j + 1) * 512],
                                         start=first,
                                         stop=(b == B - 1 and h == 1))
                        first = False
                ssl = slice(g * 1024 + j * 512, g * 1024 + (j + 1) * 512)
                nc.vector.tensor_tensor(scores[:, ssl], pch[:],
                                        fm_sb[:, ssl], op=Alu.add)
                nc.vector.tensor_reduce(mparts[:, g * 2 + j: g * 2 + j + 1],
                                        scores[:, ssl],
                                        axis=mybir.AxisListType.X, op=Alu.max)

        # ---- softmax (kv col killed by fm; new token via rank-1) ----
        m1 = io.tile([16, 1], f32, tag="m1")
        nc.vector.tensor_reduce(m1[:], mparts[:], axis=mybir.AxisListType.X,
                                op=Alu.max)
        tmax = io.tile([16, 1], f32, tag="tmax")
        nc.vector.tensor_tensor(tmax[:], m1[:], s_new[:], op=Alu.max)
        negmax = io.tile([16, 1], f32, tag="negmax")
        nc.vector.tensor_scalar_mul(negmax[:], tmax[:], -1.0)
        sumz = io.tile([16, 1], f32, tag="sumz")
        nc.scalar.activation(scores[:], scores[:], Act.Exp, bias=negmax[:],
                             accum_out=sumz[:])
        p_kv = io.tile([16, 1], f32, tag="pkv")
        nc.scalar.activation(p_kv[:], s_new[:], Act.Exp, bias=negmax[:])
        norm = io.tile([16, 1], f32, tag="norm")
        nc.vector.tensor_tensor(norm[:], sumz[:], p_kv[:], op=Alu.add)
        rnorm = io.tile([16, 1], f32, tag="rnorm")
        nc.vector.reciprocal(rnorm[:], norm[:])
        # rnB[128,16]: rnorm broadcast down partitions (for end scaling)
        prt = ps.tile([1, 16], f32, tag="ps")
        nc.tensor.transpose(prt[:], rnorm[:], id_sb[:16, :16])
        rnT = io.tile([1, 16], f32, tag="rnT")
        nc.scalar.copy(rnT[:], prt[:])
        prb = ps.tile([128, 16], f32, tag="ps")
        nc.tensor.matmul(prb[:], on_sb[:], rnT[:], start=True, stop=True)
        rnB = io.tile([128, 16], f32, tag="rnB")
        nc.scalar.copy(rnB[:], prb[:])

        # probsT via PE transpose: 32 x [16,128] -> [128,16]
        probsT = io.tile([128, 32 * 16], f32, tag="probsT")
        for ct in range(32):
            pt = ps.tile([128, 16], f32, tag="ps")
            nc.tensor.transpose(pt[:], scores[:, ct * 128:(ct + 1) * 128],
                                id_sb[:16, :16])
            nc.scalar.copy(probsT[:, ct * 16:(ct + 1) * 16], pt[:])

        # selP[b', 2b+r] = delta(b',b) * pkvn[2b+r]
        pnt = ps.tile([1, 16], f32, tag="ps")
        nc.tensor.transpose(pnt[:], p_kv[:], id_sb[:16, :16])
        pkvnT = io.tile([1, 16], f32, tag="pkvnT")
        nc.scalar.copy(pkvnT[:], pnt[:])
        pob = ps.tile([B, 16], f32, tag="ps")
        nc.tensor.matmul(pob[:], on_sb[:, 0:B], pkvnT[:], start=True, stop=True)
        pkvB = io.tile([B, 16], f32, tag="pkvB")
        nc.scalar.copy(pkvB[:], pob[:])
        selP = io.tile([B, 16], f32, tag="selP")
        nc.vector.tensor_tensor(selP[:], dup_sb[:], pkvB[:], op=Alu.mult)

        # ---- phase 3: avT_{b,h2}[128(d),2(r)] = sum_ct V_ct^T @ probsT ----
        aTt = [io.tile([128, B], f32, tag=f"aT{t}", name=f"aT{t}")
               for t in range(4)]
        for b in range(B):
            vts = []
            for q in range(8):
                vtile = kvp.tile([128, 1024], f32, tag="kv", name=f"v{b}_{q}")
                nc.sync.dma_start(vtile[:], vv[b, q])
                vts.append(vtile)
            for h2 in range(2):
                pav = ps.tile([128, 2], f32, tag="ps")
                for q in range(8):
                    for sl in range(4):
                        ct = q * 4 + sl
                        nc.tensor.matmul(
                            pav[:],
                            vts[q][:, sl * 256 + h2 * 128:
                                   sl * 256 + (h2 + 1) * 128],
                            probsT[:, ct * 16 + 2 * b: ct * 16 + 2 * b + 2],
                            start=(ct == 0), stop=False)
                nc.tensor.matmul(pav[:], vn_row[h2][:],
                                 selP[:, 2 * b:2 * b + 2],
                                 start=False, stop=True)
                for r in range(2):
                    nc.vector.tensor_tensor(
                        aTt[r * 2 + h2][:, b:b + 1], pav[:, r:r + 1],
                        rnB[:, 2 * b + r: 2 * b + r + 1], op=Alu.mult)

        # ---- phase 4: y = aT.T @ W_out_shard ----
        y_sb = io.tile([B, DIM], f32, tag="ysb")
        pys = [ps.tile([B, 512], f32, tag="ps", name=f"py{n}")
               for n in range(6)]
        for t in range(4):
            wt2 = wop.tile([128, DIM], f32, tag="wo")
            nc.sync.dma_start(wt2[:], wout[t])
            for nch in range(6):
                nc.tensor.matmul(pys[nch][:], aTt[t][:],
                                 wt2[:, nch * 512:(nch + 1) * 512],
                                 start=(t == 0), stop=(t == 3))
        for nch in range(6):
            nc.scalar.copy(y_sb[:, nch * 512:(nch + 1) * 512], pys[nch][:])
        nc.sync.dma_start(y, y_sb[:])

    nc.compile()
    return nc


_CACHED = {}


def _get_bass():
    if "nc" not in _CACHED:
        _CACHED["nc"] = build_bass()
    return _CACHED["nc"]


def _prep_inputs(x, freqs_cos, freqs_sin, kv, k_cache, v_cache, mask,
                 W_qkv, W_out):
    x2 = np.asarray(x, np.float32).reshape(B, DIM)
    xT192 = np.ascontiguousarray(
        x2.T.reshape(24, 128, B).transpose(1, 0, 2).reshape(128, 24 * B))
    cos = np.asarray(freqs_cos, np.float32)[0]
    sin = np.asarray(freqs_sin, np.float32)[0]
    cs4 = np.ascontiguousarray(
        np.stack([cos * SCALE, sin * SCALE, cos, sin], 1), np.float32)
    kvp = int(np.asarray(kv).reshape(-1)[0])
    maskr = np.asarray(mask, np.float32)
    fm = np.tile(maskr, (16, 1)).astype(np.float32)
    fm[:, kvp] -= 1e30
    mkv = np.full((16, 1), maskr[0, kvp], np.float32)
    ident = np.eye(128, dtype=np.float32)
    dupm = np.zeros((B, 16), np.float32)
    for b in range(B):
        dupm[b, 2 * b] = 1.0
        dupm[b, 2 * b + 1] = 1.0
    cmask = np.zeros((128, B * 16), np.float32)
    for b in range(B):
        cmask[:, b * 16 + 2 * b] = 1.0
        cmask[:, b * 16 + 2 * b + 1] = 1.0
    ones8 = np.ones((1, 128), np.float32)
    kc = np.asarray(k_cache, np.float32)
    vc = np.asarray(v_cache, np.float32)
    Wq = np.asarray(W_qkv, np.float32)
    Wo = np.asarray(W_out, np.float32)

    in_maps = []
    for m in range(NCORES):
        wq_shard = np.concatenate([
            Wq[:, 2 * m * HD:(2 * m + 2) * HD],
            Wq[:, HQ * HD + m * HD: HQ * HD + (m + 1) * HD],
            Wq[:, (HQ + HKV) * HD + m * HD: (HQ + HKV) * HD + (m + 1) * HD],
        ], axis=1)
        wq_shard = np.ascontiguousarray(wq_shard).reshape(24, 128, 1024)
        kTs = np.ascontiguousarray(
            kc[:, :, m, :].transpose(0, 2, 1)).reshape(B, 2, 128, C)
        vsh = np.ascontiguousarray(
            vc[:, :, m, :].reshape(B, 8, 4, 128, HD).transpose(0, 1, 3, 2, 4)
        ).reshape(B, 8, 128, 1024)
        wo_shard = np.ascontiguousarray(
            Wo[m * 2 * HD:(m + 1) * 2 * HD, :]).reshape(4, 128, DIM)
        in_maps.append({
            "xT": xT192, "wqkv": wq_shard, "kT": kTs, "vv": vsh,
            "wout": wo_shard, "fm": fm, "cs4": cs4, "ident": ident,
            "mkv": mkv, "dup": dupm, "cmask": cmask, "ones8": ones8,
        })
    return in_maps


def _run(inputs, trace=False):
    from concourse.bass_utils import run_bass_kernel_spmd
    nc = _get_bass()
    in_maps = _prep_inputs(**inputs)
    res = run_bass_kernel_spmd(nc, in_maps, core_ids=list(range(NCORES)),
                               trace=trace)
    parts = [r["y"] for r in res.results]
    out = np.sum(np.stack(parts, 0), 0, dtype=np.float32)
    return out.reshape(B, S, DIM), res


def kernel(**inputs):
    out, _ = _run(inputs, trace=False)
    return out



# revision 10
# speedup vs baseline: 3.2630x; 3.2630x over previous
"""TP-8 decode attention kernel for TRN2 (Bass/Tile), bf16 compute.

Shards the 8 KV heads (2 q heads each) across 8 NeuronCores. Host
pre-casts weights and KV cache to bf16 (HBM traffic 86.5 -> 43.3 MB per
core) and lays every tensor out in the exact tile order the kernel
consumes, so all big DMAs are contiguous and >= 1 MiB.

Per core: qkv projection (lhsT = xT k-tiles [128,8], rhs = Wq bf16
streamed N=512), RoPE on DVE, PE transposes to head-major qT/kT, scores
with q stationary ([128,16] lhsT, K streamed N=512, all-batch rows with
per-batch extraction at the PSUM drain), fused softmax (exp in place
with accumulated sum, probs pre-scaled by 1/norm so the A.V drain is a
plain copy), A.V with per-batch M=2 probsT columns vs V tiles (N=256),
new-token fixup as one extra rank-8 matmul per batch, out-proj partial
(lhsT = A^T tiles [128,8]). Host sums the 8 partial outputs.
"""

import sys

sys.path.insert(0, "/opt/trn_rl_repo")

import numpy as np
import ml_dtypes

B, S, C = 8, 1, 4096
DIM = 3072
HQ, HKV, HD = 16, 8, 256
NCORES = 8
SCALE = HD ** (-0.5)
BF = ml_dtypes.bfloat16

# packed f32 constant-block column offsets
_FM, _CS, _MKV, _DUPA, _DUPB, _IDF, _ONES = 0, 4096, 4608, 4609, 4617, 4633, 4649
_CSTW = 4657


def build_bass():
    import concourse.bass as bass  # noqa: F401
    import concourse.mybir as mybir
    import concourse.tile as tile
    from concourse import bacc
    from contextlib import ExitStack

    f32 = mybir.dt.float32
    bf16 = mybir.dt.bfloat16
    Alu = mybir.AluOpType
    Act = mybir.ActivationFunctionType

    nc = bacc.Bacc("TRN2", target_bir_lowering=False, debug=False,
                   num_devices=NCORES)

    xT = nc.dram_tensor("xT", [128, 24 * B], bf16, kind="ExternalInput").ap()
    wq = nc.dram_tensor("wq", [6, 128, 4096], bf16, kind="ExternalInput").ap()
    kt = nc.dram_tensor("kt", [B, 128, 8192], bf16, kind="ExternalInput").ap()
    vt = nc.dram_tensor("vt", [B, 128, 8192], bf16, kind="ExternalInput").ap()
    wo = nc.dram_tensor("wo", [128, 4 * DIM], bf16, kind="ExternalInput").ap()
    cst = nc.dram_tensor("cst", [16, _CSTW], f32, kind="ExternalInput").ap()
    cmk = nc.dram_tensor("cmk", [128, 144], bf16, kind="ExternalInput").ap()
    y = nc.dram_tensor("y", [B, DIM], f32, kind="ExternalOutput").ap()

    with tile.TileContext(nc) as tc, ExitStack() as stk:
        sb = stk.enter_context(tc.tile_pool(name="sb", bufs=1))
        wqp = stk.enter_context(tc.tile_pool(name="wqp", bufs=2))
        kp = stk.enter_context(tc.tile_pool(name="kp", bufs=3))
        vp = stk.enter_context(tc.tile_pool(name="vp", bufs=3))
        tmp = stk.enter_context(tc.tile_pool(name="tmp", bufs=4))
        ps = stk.enter_context(tc.tile_pool(name="ps", bufs=8, space="PSUM"))

        # ---- early small DMAs ----
        xT_sb = sb.tile([128, 24 * B], bf16, tag="xT")
        nc.sync.dma_start(xT_sb[:], xT)
        cst_sb = sb.tile([16, _CSTW], f32, tag="cst")
        nc.sync.dma_start(cst_sb[:], cst)
        cmk_sb = sb.tile([128, 144], bf16, tag="cmk")
        nc.sync.dma_start(cmk_sb[:], cmk)
        cmask = cmk_sb[:, 0:128]
        idb16 = cmk_sb[0:16, 128:144]
        idb8 = cmk_sb[0:8, 128:136]
        idb2 = cmk_sb[0:2, 128:130]

        fm = cst_sb[:, _FM:_FM + C]
        cosq = cst_sb[0:8, _CS:_CS + 128]
        sinq = cst_sb[0:8, _CS + 128:_CS + 256]
        cosk = cst_sb[0:8, _CS + 256:_CS + 384]
        sink = cst_sb[0:8, _CS + 384:_CS + 512]
        mkv = cst_sb[:, _MKV:_MKV + 1]
        dupA = cst_sb[:, _DUPA:_DUPA + 8]
        dupB = cst_sb[0:8, _DUPB:_DUPB + 16]
        idf = cst_sb[:, _IDF:_IDF + 16]
        ones18 = cst_sb[0:1, _ONES:_ONES + 8]

        # ---- phase 1: qkv = x @ Wq_shard  -> psum [8, 1024] (2 banks) ----
        pq0 = ps.tile([8, 512], f32, tag="ps", name="pq0")
        pq1 = ps.tile([8, 512], f32, tag="ps", name="pq1")
        for g in range(6):
            wt = wqp.tile([128, 4096], bf16, tag="wq")
            nc.sync.dma_start(wt[:], wq[g])
            for sub in range(4):
                t = 4 * g + sub
                lhs = xT_sb[:, t * 8:(t + 1) * 8]
                nc.tensor.matmul(pq0[:], lhs, wt[:, sub * 1024:sub * 1024 + 512],
                                 start=(t == 0), stop=(t == 23))
                nc.tensor.matmul(pq1[:], lhs,
                                 wt[:, sub * 1024 + 512:sub * 1024 + 1024],
                                 start=(t == 0), stop=(t == 23))

        # ---- rope (DVE) on [8, 128] slices; outputs bf16 ----
        qrope = sb.tile([8, 512], bf16, tag="qrope")   # cols (r, half, p)
        krope = sb.tile([8, 256], bf16, tag="krope")   # cols (half, p)
        vnew = sb.tile([8, 256], bf16, tag="vnew")

        def rope(c1, c2, cosa, sina, out1, out2):
            ta = tmp.tile([8, 128], f32, tag="rt", name="ta")
            tb = tmp.tile([8, 128], f32, tag="rt", name="tb")
            nc.vector.tensor_tensor(ta[:], c1, cosa, op=Alu.mult)
            nc.vector.tensor_tensor(tb[:], c2, sina, op=Alu.mult)
            nc.vector.tensor_tensor(out1, ta[:], tb[:], op=Alu.subtract)
            tc_ = tmp.tile([8, 128], f32, tag="rt", name="tc_")
            td = tmp.tile([8, 128], f32, tag="rt", name="td")
            nc.vector.tensor_tensor(tc_[:], c1, sina, op=Alu.mult)
            nc.vector.tensor_tensor(td[:], c2, cosa, op=Alu.mult)
            nc.vector.tensor_tensor(out2, tc_[:], td[:], op=Alu.add)

        for r in range(2):
            rope(pq0[:, r * 256:r * 256 + 128], pq0[:, r * 256 + 128:(r + 1) * 256],
                 cosq, sinq,
                 qrope[:, (2 * r) * 128:(2 * r) * 128 + 128],
                 qrope[:, (2 * r + 1) * 128:(2 * r + 1) * 128 + 128])
        rope(pq1[:, 0:128], pq1[:, 128:256], cosk, sink,
             krope[:, 0:128], krope[:, 128:256])
        nc.scalar.copy(vnew[:], pq1[:, 256:512])

        # ---- transposes to qT halves [128,16] (cols 2b+r) and kT [128,8] ----
        qTh = [sb.tile([128, 16], bf16, tag=f"qTh{h}", name=f"qTh{h}")
               for h in range(2)]
        knT = [sb.tile([128, 8], bf16, tag=f"knT{h}", name=f"knT{h}")
               for h in range(2)]
        for r in range(2):
            for h in range(2):
                pt = ps.tile([128, 8], bf16, tag="ps", name=f"ptq{r}{h}")
                nc.tensor.transpose(pt[:], qrope[:, (2 * r + h) * 128:
                                                  (2 * r + h + 1) * 128],
                                    idb8)
                dst = qTh[h][:].rearrange("p (b r) -> p r b", r=2)[:, r]
                nc.scalar.copy(dst, pt[:])
        for h in range(2):
            pt = ps.tile([128, 8], bf16, tag="ps", name=f"ptk{h}")
            nc.tensor.transpose(pt[:], krope[:, h * 128:(h + 1) * 128],
                                idb8)
            nc.scalar.copy(knT[h][:], pt[:])

        # ---- s_new[16,1]: q . k_new, diag extraction ----
        psn = ps.tile([16, 8], f32, tag="ps", name="psn")
        for h in range(2):
            nc.tensor.matmul(psn[:], qTh[h][:], knT[h][:],
                             start=(h == 0), stop=(h == 1))
        snm = sb.tile([16, 8], f32, tag="snm")
        nc.vector.tensor_tensor(snm[:], psn[:], dupA, op=Alu.mult)
        s_new = sb.tile([16, 1], f32, tag="snew")
        nc.vector.tensor_reduce(s_new[:], snm[:], axis=mybir.AxisListType.X,
                                op=Alu.add)
        nc.vector.tensor_scalar_add(s_new[:], s_new[:], mkv)

        # masked q: qThM[h][:, b*16+c] = qTh[h][:, c] if c in {2b, 2b+1} else 0
        qThM = [sb.tile([128, 128], bf16, tag=f"qThM{h}", name=f"qThM{h}")
                for h in range(2)]
        for h in range(2):
            for b in range(B):
                nc.vector.tensor_tensor(qThM[h][:, b * 16:(b + 1) * 16],
                                        qTh[h][:],
                                        cmask[:, b * 16:(b + 1) * 16],
                                        op=Alu.mult)

        # ---- phase 2: scores [16, 4096]; 8 banks accumulate over batches ----
        scores = sb.tile([16, C], f32, tag="scores")
        pchs = [ps.tile([16, 512], f32, tag="ps", name=f"sc{j}")
                for j in range(8)]
        for b in range(B):
            ktile = kp.tile([128, 8192], bf16, tag="k", name=f"k{b}")
            nc.sync.dma_start(ktile[:], kt[b])
            for j in range(8):
                nc.tensor.matmul(pchs[j][:], qThM[0][:, b * 16:(b + 1) * 16],
                                 ktile[:, j * 512:(j + 1) * 512],
                                 start=(b == 0), stop=False)
                nc.tensor.matmul(pchs[j][:], qThM[1][:, b * 16:(b + 1) * 16],
                                 ktile[:, 4096 + j * 512:4096 + (j + 1) * 512],
                                 start=False, stop=(b == B - 1))
        for j in range(8):
            ssl = slice(j * 512, (j + 1) * 512)
            nc.vector.tensor_tensor(scores[:, ssl], pchs[j][:], fm[:, ssl],
                                    op=Alu.add)

        # ---- softmax (new token via s_new; probs pre-scaled by 1/norm) ----
        m1 = sb.tile([16, 1], f32, tag="m1")
        nc.vector.tensor_reduce(m1[:], scores[:], axis=mybir.AxisListType.X,
                                op=Alu.max)
        tmax = sb.tile([16, 1], f32, tag="tmax")
        nc.vector.tensor_tensor(tmax[:], m1[:], s_new[:], op=Alu.max)
        negmax = sb.tile([16, 1], f32, tag="negmax")
        nc.vector.tensor_scalar_mul(negmax[:], tmax[:], -1.0)
        sumz = sb.tile([16, 1], f32, tag="sumz")
        nc.scalar.activation(scores[:], scores[:], Act.Exp, bias=negmax[:],
                             accum_out=sumz[:])
        p_new = sb.tile([16, 1], f32, tag="pnew")
        nc.scalar.activation(p_new[:], s_new[:], Act.Exp, bias=negmax[:])
        norm = sb.tile([16, 1], f32, tag="norm")
        nc.vector.tensor_tensor(norm[:], sumz[:], p_new[:], op=Alu.add)
        rnorm = sb.tile([16, 1], f32, tag="rnorm")
        nc.vector.reciprocal(rnorm[:], norm[:])
        probs = sb.tile([16, C], bf16, tag="probs")
        nc.vector.tensor_scalar_mul(probs[:], scores[:], rnorm[:, 0:1])
        pnorm = sb.tile([16, 1], f32, tag="pnorm")
        nc.vector.tensor_tensor(pnorm[:], p_new[:], rnorm[:], op=Alu.mult)

        # probsT via PE transpose: 32 x [16,128] -> [128,16] bf16
        probsT = sb.tile([128, 32 * 16], bf16, tag="probsT")
        for ct in range(32):
            pt = ps.tile([128, 16], bf16, tag="ps", name=f"pt{ct}")
            nc.tensor.transpose(pt[:], probs[:, ct * 128:(ct + 1) * 128],
                                idb16)
            nc.scalar.copy(probsT[:, ct * 16:(ct + 1) * 16], pt[:])

        # selPT[8,16] = dupB * broadcast(pnorm^T): for new-token A.V term
        pnt = ps.tile([1, 16], f32, tag="ps", name="pnt")
        nc.tensor.transpose(pnt[:], pnorm[:], idf)
        pnT = sb.tile([1, 16], f32, tag="pnT")
        nc.scalar.copy(pnT[:], pnt[:])
        pb = ps.tile([8, 16], f32, tag="ps", name="pb")
        nc.tensor.matmul(pb[:], ones18, pnT[:], start=True, stop=True)
        selPT = sb.tile([8, 16], bf16, tag="selPT")
        nc.vector.tensor_tensor(selPT[:], pb[:], dupB, op=Alu.mult)

        # ---- phase 3: A = probs @ V per batch, M=2, N=256 ----
        aTall = sb.tile([128, 32], bf16, tag="aTall")  # cols (r, half, b)
        for b in range(B):
            vtile = vp.tile([128, 8192], bf16, tag="v", name=f"v{b}")
            nc.sync.dma_start(vtile[:], vt[b])
            pav = ps.tile([2, 256], f32, tag="ps", name=f"av{b}")
            for ct in range(32):
                nc.tensor.matmul(pav[:],
                                 probsT[:, ct * 16 + 2 * b:ct * 16 + 2 * b + 2],
                                 vtile[:, ct * 256:(ct + 1) * 256],
                                 start=(ct == 0), stop=False)
            nc.tensor.matmul(pav[:], selPT[:, 2 * b:2 * b + 2], vnew[:],
                             start=False, stop=True)
            asb = tmp.tile([2, 256], bf16, tag="asb", name=f"asb{b}")
            nc.vector.tensor_copy(asb[:], pav[:])
            for h in range(2):
                pt2 = ps.tile([128, 2], bf16, tag="ps", name=f"pat{b}{h}")
                nc.tensor.transpose(pt2[:], asb[:, h * 128:(h + 1) * 128],
                                    idb2)
                dst = aTall[:].rearrange("p (r h b) -> p h b r", r=2, h=2,
                                         b=8)[:, h, b]
                nc.scalar.copy(dst, pt2[:])

        # ---- phase 4: y = A^T tiles @ Wout_shard ----
        wo_sb = sb.tile([128, 4 * DIM], bf16, tag="wo")
        nc.sync.dma_start(wo_sb[:], wo)
        y_sb = sb.tile([B, DIM], f32, tag="ysb")
        pys = [ps.tile([8, 512], f32, tag="ps", name=f"py{n}")
               for n in range(6)]
        for t in range(4):
            for nch in range(6):
                nc.tensor.matmul(pys[nch][:], aTall[:, t * 8:(t + 1) * 8],
                                 wo_sb[:, t * DIM + nch * 512:
                                       t * DIM + (nch + 1) * 512],
                                 start=(t == 0), stop=(t == 3))
        for nch in range(6):
            nc.vector.tensor_copy(y_sb[:, nch * 512:(nch + 1) * 512],
                                  pys[nch][:])
        nc.sync.dma_start(y, y_sb[:])

    nc.compile()
    return nc


_CACHED = {}


def _get_bass():
    if "nc" not in _CACHED:
        _CACHED["nc"] = build_bass()
    return _CACHED["nc"]


def _prep_inputs(x, freqs_cos, freqs_sin, kv, k_cache, v_cache, mask,
                 W_qkv, W_out):
    x2 = np.asarray(x, np.float32).reshape(B, DIM)
    xT192 = np.ascontiguousarray(
        x2.T.reshape(24, 128, B).transpose(1, 0, 2).reshape(128, 24 * B)
    ).astype(BF)
    cos = np.asarray(freqs_cos, np.float32)[0]
    sin = np.asarray(freqs_sin, np.float32)[0]
    kvp = int(np.asarray(kv).reshape(-1)[0])
    maskr = np.asarray(mask, np.float32)

    cst = np.zeros((16, _CSTW), np.float32)
    cst[:, _FM:_FM + C] = np.tile(maskr, (16, 1))
    cst[:, _FM + kvp] -= 1e30
    cs = np.concatenate([cos * SCALE, sin * SCALE, cos, sin])
    cst[0:8, _CS:_CS + 512] = np.tile(cs, (8, 1))
    cst[:, _MKV] = maskr[0, kvp]
    for b in range(B):
        cst[2 * b, _DUPA + b] = 1.0
        cst[2 * b + 1, _DUPA + b] = 1.0
        cst[b, _DUPB + 2 * b] = 1.0
        cst[b, _DUPB + 2 * b + 1] = 1.0
    cst[:, _IDF:_IDF + 16] = np.eye(16, dtype=np.float32)
    cst[0, _ONES:_ONES + 8] = 1.0
    cmk = np.zeros((128, 144), np.float32)
    for b in range(B):
        cmk[:, b * 16 + 2 * b] = 1.0
        cmk[:, b * 16 + 2 * b + 1] = 1.0
    cmk[0:16, 128:144] = np.eye(16, dtype=np.float32)
    cmk = cmk.astype(BF)

    KB = np.asarray(k_cache, np.float32).astype(BF)   # [B, C, HKV, HD]
    VB = np.asarray(v_cache, np.float32).astype(BF)
    WqB = np.asarray(W_qkv, np.float32).astype(BF)    # [DIM, 8192]
    WoB = np.asarray(W_out, np.float32).astype(BF)    # [4096, DIM]

    in_maps = []
    for m in range(NCORES):
        wq_shard = np.concatenate([
            WqB[:, 2 * m * HD:(2 * m + 2) * HD],
            WqB[:, HQ * HD + m * HD: HQ * HD + (m + 1) * HD],
            WqB[:, (HQ + HKV) * HD + m * HD: (HQ + HKV) * HD + (m + 1) * HD],
        ], axis=1)                                     # [3072, 1024]
        wq6 = np.ascontiguousarray(
            wq_shard.reshape(6, 4, 128, 1024).transpose(0, 2, 1, 3)
        ).reshape(6, 128, 4096)
        kts = np.ascontiguousarray(
            KB[:, :, m, :].transpose(0, 2, 1).reshape(B, 2, 128, C)
            .transpose(0, 2, 1, 3)
        ).reshape(B, 128, 8192)
        vts = np.ascontiguousarray(
            VB[:, :, m, :].reshape(B, 32, 128, HD).transpose(0, 2, 1, 3)
        ).reshape(B, 128, 8192)
        wo4 = np.ascontiguousarray(
            WoB[2 * m * HD:(2 * m + 2) * HD, :].reshape(2, 2, 128, DIM)
            .transpose(2, 0, 1, 3)
        ).reshape(128, 4 * DIM)
        in_maps.append({
            "xT": xT192, "wq": wq6, "kt": kts, "vt": vts, "wo": wo4,
            "cst": cst, "cmk": cmk,
        })
    return in_maps


def _run(inputs, trace=False):
    from concourse.bass_utils import run_bass_kernel_spmd
    nc = _get_bass()
    in_maps = _prep_inputs(**inputs)
    res = run_bass_kernel_spmd(nc, in_maps, core_ids=list(range(NCORES)),
                               trace=trace)
    parts = [r["y"] for r in res.results]
    out = np.sum(np.stack(parts, 0), 0, dtype=np.float32)
    return out.reshape(B, S, DIM), res


def kernel(**inputs):
    out, _ = _run(inputs, trace=False)
    return out
